# revision 50
# baseline (speedup 1.0000x reference)
"""Trainium2 Bass kernel for masked causal multi-head attention.

Problem: B=2, T=2048, C=1024, H=16 heads, D=64. Causal + padding mask.

Sharding (8 cores): core = 4*b + g handles batch b and head group g
(4 heads). Each core computes its qkv projection slice, attention for
its 4 heads, and a partial output projection (row slice of w_out).
Host unshard: out[b] = sum_g partial[4b+g] + b_out * m[b].

Per-core kernel (all matmuls bf16, f32 accumulation):
  Scores are computed transposed (S^T, keys on partitions) so softmax
  reduction over keys rides the AV matmul: column 64 of the augmented
  V matrix holds the padding mask m_j, making its accumulated row the
  exact softmax denominator (no max-subtraction needed: scores are
  bounded for this data). V rows of padded keys are zeroed, so no
  other padding handling is required; padded query rows are masked on
  the host. Causal masking applies a gpsimd affine_select (keep i>=j,
  else 0) in place on diagonal tiles only; fully-masked i-ranges of
  diagonal tiles are never computed (subranged matmul/exp).

  The two heads of a head-pair score concurrently in the PE array via
  row tiling (contract=64 each, auto tile_position (0,0)/(64,0)).

  Schedule: the ACT engine (exp) carries ~84us of irreducible work and
  PE ~98us; the schedule starts the exp stream as early as possible
  and keeps it dense. Warm-up matmuls on a never-written junk tile are
  the first PE instructions (HAM un-throttles during the load phase);
  weight loads are split by contraction half across the three
  DMA-capable queues; only the six B(0) units attention(0) needs run
  before the attention stream starts. Per tile-slot the emission order
  is score(t) -> exp(t) -> fillers -> AV(t-1): the AV's wait on
  exp(t-1) lands after the fillers, so the next slot's score pair
  reaches the PE right when its PSUM buffer frees and the exp stream
  never bubbles. Remaining qkv chunks / output projections are split
  into <=0.9us micro-steps paced evenly across each phase's slots.

  Softmax normalization: reciprocal of the denominator row (PSUM row 64)
  is partition-broadcast by a stride-0 DMA round trip (gpsimd queue);
  the normalize multiply is deferred into the next head-pair's stream.
  The final boundary broadcasts via a ones-stationary matmul on the
  (idle) PE.

Layouts (partition dim first):
  xT   (128, 8, 2048)  x transposed (host-side), bf16
  qT/kT (128, 2, 2048) head-channel rows, bf16
  V    (128, 16, 4, 65) [j-tile, head, 64 V cols | m_j], bf16
  S^T  (128 j, 2 s, 512 i) per j-tile; exp'd P^T batched 2 heads wide
  aoT  (128, 2, 2048)  attention out, channel-major, bf16
"""

import numpy as np
import ml_dtypes

import concourse.bass as bass  # noqa: F401  (engine types)
import concourse.mybir as mybir
import concourse.tile as tile
from concourse import bacc
from concourse.masks import make_identity
from concourse.bass_utils import run_bass_kernel_spmd

P = 128
T = 2048
C = 1024
NH = 16          # total heads
D = 64
LH = 4           # heads per core
LC = LH * D      # 256 local channels
CC = C // P      # 8 contract chunks
NTT = T // P     # 16 t-tiles
NIC = 4          # i-chunks of 512
ICW = 512
SCALE = D ** -0.5

dt32 = mybir.dt.float32
dtb = mybir.dt.bfloat16
MM = mybir.ActivationFunctionType
ALU = mybir.AluOpType


def ts(i, n):
    return slice(i * n, (i + 1) * n)


def build():
    nc = bacc.Bacc("TRN2", target_bir_lowering=False, debug=False)
    # all inputs arrive host-repacked so every DMA reads fully
    # contiguous 2-8KB lines per partition (the naive rearrange loads
    # measured ~40GB/s/queue on 512B lines and starved the start-up).
    xt_ext = nc.declare_dram_parameter("xt", [P, NIC, CC, ICW], dtb,
                                       isOutput=False)
    wq_ext = nc.declare_dram_parameter("wq", [P, CC, LC], dtb, isOutput=False)
    wk_ext = nc.declare_dram_parameter("wk", [P, CC, LC], dtb, isOutput=False)
    wv_ext = nc.declare_dram_parameter("wv", [P, CC, LC], dtb, isOutput=False)
    wo_ext = nc.declare_dram_parameter("wo", [P, 2, C], dtb, isOutput=False)
    m_ext = nc.declare_dram_parameter("m", [T], dt32, isOutput=False)
    out_ext = nc.declare_dram_parameter("out", [T, C], dtb, isOutput=True)

    out_r = out_ext[:].rearrange("(n p) c -> n p c", p=P)

    with tile.TileContext(nc) as tc:
        with (
            tc.tile_pool(name="const", bufs=1) as cpool,
            tc.tile_pool(name="big", bufs=1) as big,
            tc.tile_pool(name="stage", bufs=4) as stage,
            tc.tile_pool(name="dram", bufs=4, space="DRAM") as dram_pool,
            tc.tile_pool(name="psB", bufs=2, space="PSUM") as psB,
            tc.tile_pool(name="psC", bufs=1, space="PSUM") as psC,
            tc.tile_pool(name="psPT", bufs=2, space="PSUM") as psPT,
        ):
            # ---------------- persistent tiles -----------------------
            ident = cpool.tile([P, P], dtb)
            ones_row = cpool.tile([1, 64], dtb)
            junk = cpool.tile([P, P], dtb)       # never written: warm-up fuel
            msc = cpool.tile([P, 16], dt32)
            warm_act = cpool.tile([1, 64], dt32)

            xT = big.tile([P, NIC, CC, ICW], dtb)  # t-chunk major
            qT = big.tile([P, 2, T], dtb)
            kT = big.tile([P, 2, T], dtb)
            v_sb = big.tile([P, NTT, LH, 65], dtb)
            aoT = big.tile([P, 2, T], dtb)

            wq_sb = big.tile([P, CC, LC], dtb)
            wk_sb = big.tile([P, CC, LC], dtb)
            wv_sb = big.tile([P, CC, LC], dtb)
            wo_sb = big.tile([P, 2, C], dtb)

            # ---------------- warm-up + preloads ---------------------
            # PE: a matmul burst gated only on a tiny gpsimd memset
            # (~3.5us at cold clock) is the very first thing in the PE
            # queue, so the HAM clock gate reaches 8/8 while the loads
            # stream.
            nc.gpsimd.memset(junk[:], 0.0)
            warm_ps = psB.tile([P, ICW], dt32, tag="bps", name="warm_ps")
            for _w in range(88):
                nc.tensor.matmul(
                    warm_ps[:, 0:P], junk[:], junk[:],
                    start=True, stop=True,
                )
            # ACT: preload the exp spline table (first real exp then
            # pays no ACT_TABLE_LOAD).
            nc.gpsimd.memset(ones_row[:], 1.0)
            nc.scalar.activation(warm_act[:], ones_row[:], MM.Exp, scale=1.0)
            make_identity(nc, ident[:])

            # ---------------- DMA plan ------------------------------
            # DMA-capable queues: sync, scalar, gpsimd. scalar/gpsimd
            # only carry early loads (the exp stream owns scalar from
            # ~10us, affine_selects own gpsimd); weight loads are split
            # by contraction half so the first projection chains start
            # as soon as possible. t-chunks 2-3 are dispatched later as
            # paced filler steps inside attention(0)/(1).
            # Only the B(0)-critical 2MB loads up front: a larger burst
            # (these contiguous loads sustain ~350GB/s across the 16
            # shared DMA engines) trips the chip power limiter and
            # downclocks every engine by ~1/6 for the rest of the
            # kernel. Everything else trickles in as paced filler-step
            # dispatches.
            # critical 2MB spread so each FIFO ring (~80GB/s at the
            # contended start) delivers its pieces in consumption order:
            # the q/k chains pipeline behind the arriving chunks.
            nc.sync.dma_start(wq_sb[:, 0:4], wq_ext[:, 0:4])
            nc.scalar.dma_start(xT[:, 0, 0:4], xt_ext[:, 0, 0:4])
            nc.gpsimd.dma_start(xT[:, 0, 4:8], xt_ext[:, 0, 4:8])
            nc.sync.dma_start(wk_sb[:, 0:4], wk_ext[:, 0:4])
            nc.scalar.dma_start(wq_sb[:, 4:8], wq_ext[:, 4:8])
            nc.gpsimd.dma_start(wk_sb[:, 4:8], wk_ext[:, 4:8])
            m_st = stage.tile([16, P], dt32)
            nc.sync.dma_start(m_st[:], m_ext[:].rearrange("(o p) -> o p", p=P))

            def x_dma_step(tch, half):
                nc.sync.dma_start(
                    xT[:, tch, ts(half, 4)], xt_ext[:, tch, ts(half, 4)])

            def wv_dma_step():
                nc.gpsimd.dma_start(wv_sb[:], wv_ext[:])

            def wo_dma_step(kc):
                nc.sync.dma_start(wo_sb[:, kc], wo_ext[:, kc])

            # padding mask, transposed to partition-major (128, 16).
            # The transpose rides a psB-pool buffer (f32, matching the
            # tag's size) so no extra PSUM bank is needed.
            mb_st = stage.tile([16, P], dtb)
            nc.vector.tensor_copy(mb_st[:], m_st[:])
            mt_ps = psB.tile([P, 2 * ICW], dtb, tag="bps", name="mt_ps")
            nc.tensor.transpose(mt_ps[:, 0:16], mb_st[:], ident[:16, :16])
            nc.vector.tensor_copy(msc[:], mt_ps[:, 0:16])

            # column 64 of each V tile = m_j: its accumulated row is the
            # softmax denominator (padded keys excluded exactly).
            for h in range(LH):
                nc.vector.tensor_copy(v_sb[:, :, h, 64:65], msc[:, :, None])

            # ---- B-phase unit bodies --------------------------------
            def qk_emit(w_sb, dstT, ch, tch, half):
                """Half of a q/k projection chain (4 of 8 cc chunks)."""
                if half == 0:
                    t = psB.tile([P, ICW], dt32, tag="bps", name="qk_ps")
                    qk_emit.live[(id(w_sb), ch, tch)] = t
                else:
                    t = qk_emit.live.pop((id(w_sb), ch, tch))
                for cc in range(4 * half, 4 * half + 4):
                    nc.tensor.matmul(
                        t[:],
                        w_sb[:, cc, ts(ch, P)],
                        xT[:, tch, cc],
                        start=(cc == 0), stop=(cc == CC - 1),
                    )
                if half == 1:
                    if tch == 0 and ch == 0:
                        # B(0) copybacks ride the Scalar engine: it is
                        # idle before the first exp, while the DVE queue
                        # still holds the mask/v-column setup — shaves
                        # the critical path to the first score pair.
                        nc.scalar.copy(dstT[:, ch, ts(tch, ICW)], t[:])
                    else:
                        nc.vector.tensor_copy(dstT[:, ch, ts(tch, ICW)], t[:])
            qk_emit.live = {}

            def v_emit(tt):
                tch, o = divmod(tt, 4)
                v_ps = psB.tile([P, LC], dt32, tag="bps", name="v_ps")
                for cc in range(CC):
                    nc.tensor.matmul(
                        v_ps[:],
                        xT[:, tch, cc, ts(o, P)],
                        wv_sb[:, cc, :],
                        start=(cc == 0), stop=(cc == CC - 1),
                    )
                # zero padded value rows while copying back
                nc.vector.tensor_scalar_mul(
                    v_sb[:, tt, :, 0:64],
                    v_ps[:].rearrange("p (h d) -> p h d", h=LH),
                    msc[:, tt:tt + 1],
                )

            def op_emit(ic, o, last=False):
                # both column halves of one t-tile, so the store is a
                # single DMA with 2KB DRAM lines.
                tt = ic * 4 + o
                ot = stage.tile([P, C], dtb, tag="ot", name="ot")
                for ncol in range(2):
                    op_ps = psB.tile([P, ICW], dt32, tag="bps", name="op_ps")
                    for kc in range(2):
                        nc.tensor.matmul(
                            op_ps[:],
                            aoT[:, kc, ts(tt, P)],
                            wo_sb[:, kc, ts(ncol, ICW)],
                            start=(kc == 0), stop=(kc == 1),
                        )
                    if last:
                        # tail chunks: the Scalar engine is idle after
                        # the final exp and PSUM is its fast path.
                        nc.scalar.copy(ot[:, ts(ncol, ICW)], op_ps[:])
                    else:
                        nc.vector.tensor_copy(ot[:, ts(ncol, ICW)], op_ps[:])
                eng = (nc.sync, nc.gpsimd)[o % 2] if last else nc.sync
                eng.dma_start(out_r[tt][:], ot[:])

            import functools as _ft

            def qk_ch_steps(tch, ch):
                """One channel chunk's q/k chains as ~0.9us micro-steps.
                Channel 1 is only read from att(tch) hp=1, so it can
                ride one phase later than channel 0."""
                out = []
                for wsb, dstT in ((wq_sb, qT), (wk_sb, kT)):
                    for half in range(2):
                        out.append((
                            850,
                            _ft.partial(qk_emit, wsb, dstT, ch,
                                        tch, half)))
                return out

            def v_steps(tch):
                return [(860, _ft.partial(v_emit, tch * 4 + o))
                        for o in range(4)]

            def op_steps(ic):
                return [(860, _ft.partial(op_emit, ic, o))
                        for o in range(4)]

            # ---------------- attention --------------------------
            deferred = []

            def attention(ic, fillers):
                njt = (ic + 1) * 4
                nslots = 2 * njt
                fillers = list(fillers)
                # normalize suffixes deferred from the previous chunk
                # run first (out-projection fillers read their aoT).
                for fn in deferred:
                    # a few slots in: the reciprocal-broadcast DMA the
                    # suffix reads must land first, or its wait blocks
                    # the DVE queue behind it.
                    fillers.insert(min(4, len(fillers)), (200, fn))
                deferred.clear()
                fill_total = sum(n for n, _ in fillers) or 1.0
                state = {"slot": 0, "fill": 0.0}

                def run_next():
                    n, fn = fillers.pop(0)
                    state["fill"] += n
                    fn()

                def pop_fillers():
                    state["slot"] += 1
                    target = fill_total * state["slot"] / nslots
                    while fillers and state["fill"] < target:
                        run_next()

                for hp in range(2):       # head pair = channel chunk
                    o_ps = [
                        psC.tile([65, ICW], dt32, tag=f"o{s}",
                                 name=f"o_ps{s}")
                        for s in range(2)
                    ]
                    pending_av = None
                    for jt in range(njt):
                        # diagonal tiles: only i >= j is reachable;
                        # skip the fully-masked left part.
                        r = jt - ic * 4
                        off = max(r, 0) * P
                        pt_ps = psPT.tile(
                            [P, 2 * ICW], dt32, tag="pt", name="pt_ps")
                        pt_sb = stage.tile(
                            [P, 2 * ICW], dtb, tag="pt_sb", name="pt_sb")
                        # the two heads of the pair run concurrently in
                        # the PE array (row tiling: contract=64 each).
                        for s in range(2):
                            nc.tensor.matmul(
                                pt_ps[:, s * ICW + off:(s + 1) * ICW],
                                kT[ts(s, 64), hp, ts(jt, P)],
                                qT[ts(s, 64), hp,
                                   ic * ICW + off:(ic + 1) * ICW],
                                start=True, stop=True,
                            )
                        pt_ps3 = pt_ps[:].rearrange("p (s w) -> p s w", s=2)
                        pt_sb3 = pt_sb[:].rearrange("p (s w) -> p s w", s=2)
                        nc.scalar.activation(
                            pt_sb3[:, :, off:], pt_ps3[:, :, off:],
                            MM.Exp, scale=SCALE,
                        )
                        if r >= 0:
                            # causal tri mask on the diagonal block, in
                            # place on gpsimd: keep i >= j, else 0
                            for s in range(2):
                                nc.gpsimd.affine_select(
                                    out=pt_sb[:, s * ICW + off:
                                              s * ICW + off + P],
                                    in_=pt_sb[:, s * ICW + off:
                                              s * ICW + off + P],
                                    compare_op=ALU.is_ge, fill=0.0,
                                    base=0, pattern=[[1, P]],
                                    channel_multiplier=-1,
                                )
                        # fillers first: they are dependency-free, so the
                        # PE stays busy while exp(jt-1) finishes; only
                        # then the AV whose wait would block the queue.
                        pop_fillers()
                        if pending_av is not None:
                            pending_av()

                        def av(jt=jt, off=off, pt_sb=pt_sb, hp=hp,
                               o_ps=o_ps, njt=njt):
                            for s in range(2):
                                h = 2 * hp + s
                                nc.tensor.matmul(
                                    o_ps[s][:, off:],
                                    v_sb[:, jt, h, :],
                                    pt_sb[:, s * ICW + off:(s + 1) * ICW],
                                    start=(jt == 0),
                                    stop=(jt == njt - 1),
                                )
                        pending_av = av
                    pending_av()

                    # boundary: evacuate the AV accumulators into SBUF
                    # staging right away (frees the PSUM banks for the
                    # next head-pair's AVs) and compute the reciprocals;
                    # the broadcast + normalize multiply are DEFERRED
                    # into the next stream so the PE FIFO never stalls
                    # on the DVE chain.
                    final_hp = ic == 3 and hp == 1
                    sts, recbs = [], []
                    for s in range(2):
                        if not final_hp:
                            st = stage.tile(
                                [P, ICW], dt32, tag=f"st{s}", name="st")
                            nc.vector.tensor_copy(
                                st[ts(s, 64), :], o_ps[s][0:64, :])
                            sts.append(st)
                        # denominator row to SBUF, reciprocal on DVE.
                        # At the final boundary the exp stream is done,
                        # so s=1's copy rides the idle Scalar engine and
                        # the serial DVE chain shortens.
                        den = stage.tile(
                            [1, ICW], dt32, tag="den", name="den")
                        if final_hp and s == 1:
                            nc.scalar.copy(den[:], o_ps[s][64:65, :])
                        else:
                            nc.vector.tensor_copy(den[:], o_ps[s][64:65, :])
                        rec = stage.tile(
                            [1, ICW], dt32, tag="rec", name="rec")
                        nc.vector.reciprocal_approx_fast(rec[:], den[:])
                        recb = stage.tile(
                            [1, ICW], dtb, tag=f"recb{s}", name="recb")
                        nc.vector.tensor_copy(recb[:], rec[:])
                        recbs.append(recb)

                    if final_hp:
                        # final boundary: drain whatever fillers remain
                        # (the hp=0 suffix among them), then broadcast
                        # the reciprocals with ones-matmuls and
                        # normalize in 128-col chunks, each feeding its
                        # output-projection unit immediately so the tail
                        # pipeline (DVE mul -> PE matmuls -> ACT copy ->
                        # DMA) stays full.
                        while fillers:
                            run_next()
                        # junk-matmul bridge on a psPT-pool tile (its
                        # previous user finished long ago, so no WAR
                        # delay): keeps the HAM clock warm through the
                        # ~3us reciprocal chain so the bc matmuls and
                        # out-projections run at full clock.
                        br_ps = psPT.tile([P, 2 * ICW], dt32, tag="pt",
                                          name="br_ps")
                        for _w in range(40):
                            nc.tensor.matmul(
                                br_ps[:, 0:P], junk[:], junk[:],
                                start=True, stop=True,
                            )
                        bcs_f = []
                        for s in range(2):
                            bc_ps = psPT.tile(
                                [P, 2 * ICW], dt32, tag="pt",
                                name="bc_fin")
                            nc.tensor.matmul(
                                bc_ps[0:64, 0:ICW], ones_row[:],
                                recbs[s][:], start=True, stop=True,
                            )
                            bc_sb = stage.tile(
                                [P, ICW], dtb, tag=f"bc{s}", name="bc_sb")
                            nc.vector.tensor_copy(
                                bc_sb[ts(s, 64), :], bc_ps[0:64, 0:ICW])
                            bcs_f.append(bc_sb)
                        # second bridge stretch: the broadcast casts and
                        # first normalize muls are ~3us of DVE before the
                        # first out-projection matmul can start.
                        for _w in range(56):
                            nc.tensor.matmul(
                                br_ps[:, 0:P], junk[:], junk[:],
                                start=True, stop=True,
                            )
                        for o in range(4):
                            cs = slice(o * P, (o + 1) * P)
                            for s in range(2):
                                ao_slice = aoT[ts(s, 64), hp,
                                               ic * ICW + o * P:
                                               ic * ICW + (o + 1) * P]
                                if s == 0:
                                    nc.vector.tensor_mul(
                                        ao_slice, o_ps[s][0:64, cs],
                                        bcs_f[s][0:64, cs])
                                else:
                                    nc.vector.tensor_copy(
                                        ao_slice, o_ps[s][0:64, cs])
                                    nc.vector.tensor_mul(
                                        ao_slice, ao_slice,
                                        bcs_f[s][64:128, cs])
                            op_emit(3, o, last=True)
                    else:
                        # partition-broadcast by a stride-0 DMA round
                        # trip on the (lightly loaded) gpsimd queue,
                        # issued NOW so the transfer overlaps the
                        # deferred window.
                        bcs = []
                        for s in range(2):
                            rec_d = dram_pool.tile(
                                [1, ICW], dtb, name="rec_d")
                            nc.gpsimd.dma_start(rec_d[:], recbs[s][:])
                            bc_sb = stage.tile(
                                [P, ICW], dtb, tag=f"bc{s}", name="bc_sb")
                            nc.gpsimd.dma_start(
                                bc_sb[ts(s, 64), :],
                                rec_d[0:1, :].to_broadcast((64, ICW)),
                            )
                            bcs.append(bc_sb)

                        def suffix(ic=ic, hp=hp, sts=sts, bcs=bcs):
                            for s in range(2):
                                nc.vector.tensor_mul(
                                    aoT[ts(s, 64), hp, ts(ic, ICW)],
                                    sts[s][ts(s, 64), :],
                                    bcs[s][ts(s, 64), :],
                                )

                        if hp == 0:
                            fillers.insert(min(4, len(fillers)),
                                           (200, suffix))
                        else:
                            deferred.append(suffix)
                # drain stragglers before the next phase's stream
                while fillers:
                    run_next()

            # ---------------- top-level schedule -----------------
            # only the two q/k chains attention(0) hp=0 needs run
            # up-front; the v units ride at the head of the filler
            # stream (AV(0) only needs them one slot later), so the
            # first exp fires as early as possible.
            for half in range(2):
                qk_emit(wq_sb, qT, 0, 0, half)
            for half in range(2):
                qk_emit(wk_sb, kT, 0, 0, half)

            f0 = [(50, wv_dma_step)]
            f0 += v_steps(0)[:2]
            f0 += [(50, _ft.partial(x_dma_step, 1, 0))]
            f0 += v_steps(0)[2:]
            f0 += [(50, _ft.partial(x_dma_step, 1, 1))]
            for ch in range(1, 2):
                for wsb, dstT in ((wq_sb, qT), (wk_sb, kT)):
                    for half in range(2):
                        f0.append((850, _ft.partial(
                            qk_emit, wsb, dstT, ch, 0, half)))
            f0 += [(50, _ft.partial(x_dma_step, 2, 0)),
                   (50, _ft.partial(wo_dma_step, 0))]
            f0 += qk_ch_steps(1, 0) + v_steps(1)
            f0 += [(50, _ft.partial(x_dma_step, 2, 1)),
                   (50, _ft.partial(wo_dma_step, 1))]
            f1 = qk_ch_steps(1, 1)
            f1 += [(50, _ft.partial(x_dma_step, 3, half))
                   for half in range(2)]
            f1 += qk_ch_steps(2, 0) + v_steps(2) + op_steps(0)
            attention(0, f0)
            attention(1, f1)
            attention(2, qk_ch_steps(2, 1) + qk_ch_steps(3, 0)
                      + v_steps(3) + op_steps(1))
            attention(3, qk_ch_steps(3, 1) + op_steps(2))
    nc.finalize()
    return nc


_CACHE = {}


def _get_nc():
    if "nc" not in _CACHE:
        _CACHE["nc"] = build()
    return _CACHE["nc"]


def _pack_w(w):
    # [C, F] -> [P, CC, F]: partition-major with contiguous per-
    # partition lines so the device DMA is a straight streaming read.
    f = w.shape[1]
    return np.ascontiguousarray(w.reshape(CC, P, f).transpose(1, 0, 2))


def make_in_maps(x, m, w_qkv, w_out):
    bf = ml_dtypes.bfloat16
    in_maps = []
    for core in range(8):
        b, g = divmod(core, 4)
        xt = np.asarray(x[b]).T.astype(bf)          # [C, T]
        xt = xt.reshape(CC, P, NIC, ICW).transpose(1, 2, 0, 3)
        wo = w_out[g * LC:(g + 1) * LC, :].astype(bf)  # [LC, C]
        wo = wo.reshape(2, P, C).transpose(1, 0, 2)
        in_maps.append({
            "xt": np.ascontiguousarray(xt),          # [P, NIC, CC, ICW]
            "wq": _pack_w(w_qkv[:, g * LC:(g + 1) * LC].astype(bf)),
            "wk": _pack_w(w_qkv[:, C + g * LC: C + (g + 1) * LC].astype(bf)),
            "wv": _pack_w(w_qkv[:, 2 * C + g * LC: 2 * C + (g + 1) * LC]
                          .astype(bf)),
            "wo": np.ascontiguousarray(wo),          # [P, 2, C]
            "m": np.ascontiguousarray(m[b, :, 0]).astype(np.float32),
        })
    return in_maps


def gather(results, m, b_out, B):
    out = np.zeros((B, T, C), dtype=np.float32)
    for core in range(8):
        b = core // 4
        out[b] += results[core]["out"].astype(np.float32)
    out = (out + np.asarray(b_out)[None, None, :]) * np.asarray(m)
    return out.astype(np.float32)


def kernel(x, m, w_qkv, w_out, b_out):
    x = np.asarray(x)
    m = np.asarray(m)
    in_maps = make_in_maps(x, m, np.asarray(w_qkv), np.asarray(w_out))
    nc = _get_nc()
    res = run_bass_kernel_spmd(nc, in_maps, core_ids=list(range(8)))
    return gather(res.results, m, b_out, x.shape[0])


# revision 51
# speedup vs baseline: 1.0321x; 1.0321x over previous
"""Trainium2 Bass kernel for masked causal multi-head attention.

Problem: B=2, T=2048, C=1024, H=16 heads, D=64. Causal + padding mask.

Sharding (8 cores): core = 4*b + g handles batch b and head group g
(4 heads). Each core computes its qkv projection slice, attention for
its 4 heads, and a partial output projection (row slice of w_out).
Host unshard: out[b] = sum_g partial[4b+g] + b_out * m[b].

Per-core kernel (all matmuls bf16, f32 accumulation):
  Scores are computed transposed (S^T, keys on partitions) so softmax
  reduction over keys rides the AV matmul: column 64 of the augmented
  V matrix holds the padding mask m_j, making its accumulated row the
  exact softmax denominator (no max-subtraction needed: scores are
  bounded for this data). V rows of padded keys are zeroed, so no
  other padding handling is required; padded query rows are masked on
  the host. Causal masking applies a gpsimd affine_select (keep i>=j,
  else 0) in place on diagonal tiles only; fully-masked i-ranges of
  diagonal tiles are never computed (subranged matmul/exp).

  The two heads of a head-pair score concurrently in the PE array via
  row tiling (contract=64 each, auto tile_position (0,0)/(64,0)).

  Schedule: the ACT engine (exp) carries ~84us of irreducible work and
  PE ~98us; the schedule starts the exp stream as early as possible
  and keeps it dense. Warm-up matmuls on a never-written junk tile are
  the first PE instructions (HAM un-throttles during the load phase);
  weight loads are split by contraction half across the three
  DMA-capable queues; only the six B(0) units attention(0) needs run
  before the attention stream starts. Per tile-slot the emission order
  is score(t) -> exp(t) -> fillers -> AV(t-1): the AV's wait on
  exp(t-1) lands after the fillers, so the next slot's score pair
  reaches the PE right when its PSUM buffer frees and the exp stream
  never bubbles. Remaining qkv chunks / output projections are split
  into <=0.9us micro-steps paced evenly across each phase's slots.

  Softmax normalization: reciprocal of the denominator row (PSUM row 64)
  is partition-broadcast by a stride-0 DMA round trip (gpsimd queue);
  the normalize multiply is deferred into the next head-pair's stream.
  The final boundary broadcasts via a ones-stationary matmul on the
  (idle) PE.

Layouts (partition dim first):
  xT   (128, 8, 2048)  x transposed (host-side), bf16
  qT/kT (128, 2, 2048) head-channel rows, bf16
  V    (128, 16, 4, 65) [j-tile, head, 64 V cols | m_j], bf16
  S^T  (128 j, 2 s, 512 i) per j-tile; exp'd P^T batched 2 heads wide
  aoT  (128, 2, 2048)  attention out, channel-major, bf16
"""

import numpy as np
import ml_dtypes

import concourse.bass as bass  # noqa: F401  (engine types)
import concourse.mybir as mybir
import concourse.tile as tile
from concourse import bacc
from concourse.masks import make_identity
from concourse.bass_utils import run_bass_kernel_spmd

P = 128
T = 2048
C = 1024
NH = 16          # total heads
D = 64
LH = 4           # heads per core
LC = LH * D      # 256 local channels
CC = C // P      # 8 contract chunks
NTT = T // P     # 16 t-tiles
NIC = 4          # i-chunks of 512
ICW = 512
SCALE = D ** -0.5

dt32 = mybir.dt.float32
dtb = mybir.dt.bfloat16
MM = mybir.ActivationFunctionType
ALU = mybir.AluOpType


def ts(i, n):
    return slice(i * n, (i + 1) * n)


def build():
    nc = bacc.Bacc("TRN2", target_bir_lowering=False, debug=False)
    # all inputs arrive host-repacked so every DMA reads fully
    # contiguous 2-8KB lines per partition (the naive rearrange loads
    # measured ~40GB/s/queue on 512B lines and starved the start-up).
    xt_ext = nc.declare_dram_parameter("xt", [P, NIC, CC, ICW], dtb,
                                       isOutput=False)
    wq_ext = nc.declare_dram_parameter("wq", [P, CC, LC], dtb, isOutput=False)
    wk_ext = nc.declare_dram_parameter("wk", [P, CC, LC], dtb, isOutput=False)
    wv_ext = nc.declare_dram_parameter("wv", [P, CC, LC], dtb, isOutput=False)
    wo_ext = nc.declare_dram_parameter("wo", [P, 2, C], dtb, isOutput=False)
    m_ext = nc.declare_dram_parameter("m", [T], dt32, isOutput=False)
    out_ext = nc.declare_dram_parameter("out", [T, C], dtb, isOutput=True)

    out_r = out_ext[:].rearrange("(n p) c -> n p c", p=P)

    with tile.TileContext(nc) as tc:
        with (
            tc.tile_pool(name="const", bufs=1) as cpool,
            tc.tile_pool(name="big", bufs=1) as big,
            tc.tile_pool(name="stage", bufs=4) as stage,
            tc.tile_pool(name="dram", bufs=4, space="DRAM") as dram_pool,
            tc.tile_pool(name="psB", bufs=2, space="PSUM") as psB,
            tc.tile_pool(name="psC", bufs=1, space="PSUM") as psC,
            tc.tile_pool(name="psPT", bufs=2, space="PSUM") as psPT,
        ):
            # ---------------- persistent tiles -----------------------
            ident = cpool.tile([P, P], dtb)
            ones_row = cpool.tile([1, 64], dtb)
            junk = cpool.tile([P, P], dtb)       # never written: warm-up fuel
            msc = cpool.tile([P, 16], dt32)
            warm_act = cpool.tile([1, 64], dt32)

            xT = big.tile([P, NIC, CC, ICW], dtb)  # t-chunk major
            qT = big.tile([P, 2, T], dtb)
            kT = big.tile([P, 2, T], dtb)
            v_sb = big.tile([P, NTT, LH, 65], dtb)
            aoT = big.tile([P, 2, T], dtb)

            wq_sb = big.tile([P, CC, LC], dtb)
            wk_sb = big.tile([P, CC, LC], dtb)
            wv_sb = big.tile([P, CC, LC], dtb)
            wo_sb = big.tile([P, 2, C], dtb)

            # ---------------- warm-up + preloads ---------------------
            # PE: a matmul burst gated only on a tiny gpsimd memset
            # (~3.5us at cold clock) is the very first thing in the PE
            # queue, so the HAM clock gate reaches 8/8 while the loads
            # stream.
            nc.gpsimd.memset(junk[:], 0.0)
            warm_ps = psB.tile([P, ICW], dt32, tag="bps", name="warm_ps")
            for _w in range(88):
                nc.tensor.matmul(
                    warm_ps[:, 0:P], junk[:], junk[:],
                    start=True, stop=True,
                )
            # ACT: preload the exp spline table (first real exp then
            # pays no ACT_TABLE_LOAD).
            nc.gpsimd.memset(ones_row[:], 1.0)
            nc.scalar.activation(warm_act[:], ones_row[:], MM.Exp, scale=1.0)
            make_identity(nc, ident[:])

            # ---------------- DMA plan ------------------------------
            # DMA-capable queues: sync, scalar, gpsimd. scalar/gpsimd
            # only carry early loads (the exp stream owns scalar from
            # ~10us, affine_selects own gpsimd); weight loads are split
            # by contraction half so the first projection chains start
            # as soon as possible. t-chunks 2-3 are dispatched later as
            # paced filler steps inside attention(0)/(1).
            # Only the B(0)-critical 2MB loads up front: a larger burst
            # (these contiguous loads sustain ~350GB/s across the 16
            # shared DMA engines) trips the chip power limiter and
            # downclocks every engine by ~1/6 for the rest of the
            # kernel. Everything else trickles in as paced filler-step
            # dispatches.
            # critical 2MB spread so each FIFO ring (~80GB/s at the
            # contended start) delivers its pieces in consumption order:
            # the q/k chains pipeline behind the arriving chunks.
            nc.sync.dma_start(wq_sb[:, 0:4], wq_ext[:, 0:4])
            nc.scalar.dma_start(xT[:, 0, 0:4], xt_ext[:, 0, 0:4])
            nc.gpsimd.dma_start(xT[:, 0, 4:8], xt_ext[:, 0, 4:8])
            nc.sync.dma_start(wk_sb[:, 0:4], wk_ext[:, 0:4])
            nc.scalar.dma_start(wq_sb[:, 4:8], wq_ext[:, 4:8])
            nc.gpsimd.dma_start(wk_sb[:, 4:8], wk_ext[:, 4:8])
            m_st = stage.tile([16, P], dt32)
            nc.sync.dma_start(m_st[:], m_ext[:].rearrange("(o p) -> o p", p=P))

            def x_dma_step(tch, half):
                nc.sync.dma_start(
                    xT[:, tch, ts(half, 4)], xt_ext[:, tch, ts(half, 4)])

            def wv_dma_step():
                nc.gpsimd.dma_start(wv_sb[:], wv_ext[:])

            def wo_dma_step(kc):
                nc.sync.dma_start(wo_sb[:, kc], wo_ext[:, kc])

            # padding mask, transposed to partition-major (128, 16).
            # The transpose rides a psB-pool buffer (f32, matching the
            # tag's size) so no extra PSUM bank is needed.
            mb_st = stage.tile([16, P], dtb)
            nc.vector.tensor_copy(mb_st[:], m_st[:])
            mt_ps = psB.tile([P, 2 * ICW], dtb, tag="bps", name="mt_ps")
            nc.tensor.transpose(mt_ps[:, 0:16], mb_st[:], ident[:16, :16])
            nc.vector.tensor_copy(msc[:], mt_ps[:, 0:16])

            # column 64 of each V tile = m_j: its accumulated row is the
            # softmax denominator (padded keys excluded exactly).
            for h in range(LH):
                nc.vector.tensor_copy(v_sb[:, :, h, 64:65], msc[:, :, None])

            # ---- B-phase unit bodies --------------------------------
            def qk_emit(w_sb, dstT, ch, tch, half):
                """Half of a q/k projection chain (4 of 8 cc chunks)."""
                if half == 0:
                    t = psB.tile([P, ICW], dt32, tag="bps", name="qk_ps")
                    qk_emit.live[(id(w_sb), ch, tch)] = t
                else:
                    t = qk_emit.live.pop((id(w_sb), ch, tch))
                for cc in range(4 * half, 4 * half + 4):
                    nc.tensor.matmul(
                        t[:],
                        w_sb[:, cc, ts(ch, P)],
                        xT[:, tch, cc],
                        start=(cc == 0), stop=(cc == CC - 1),
                    )
                if half == 1:
                    nc.vector.tensor_copy(dstT[:, ch, ts(tch, ICW)], t[:])
            qk_emit.live = {}

            def v_emit(tt):
                tch, o = divmod(tt, 4)
                v_ps = psB.tile([P, LC], dt32, tag="bps", name="v_ps")
                for cc in range(CC):
                    nc.tensor.matmul(
                        v_ps[:],
                        xT[:, tch, cc, ts(o, P)],
                        wv_sb[:, cc, :],
                        start=(cc == 0), stop=(cc == CC - 1),
                    )
                # zero padded value rows while copying back
                nc.vector.tensor_scalar_mul(
                    v_sb[:, tt, :, 0:64],
                    v_ps[:].rearrange("p (h d) -> p h d", h=LH),
                    msc[:, tt:tt + 1],
                )

            def op_emit(ic, o, last=False):
                # both column halves of one t-tile, so the store is a
                # single DMA with 2KB DRAM lines.
                tt = ic * 4 + o
                ot = stage.tile([P, C], dtb, tag="ot", name="ot")
                for ncol in range(2):
                    op_ps = psB.tile([P, ICW], dt32, tag="bps", name="op_ps")
                    for kc in range(2):
                        nc.tensor.matmul(
                            op_ps[:],
                            aoT[:, kc, ts(tt, P)],
                            wo_sb[:, kc, ts(ncol, ICW)],
                            start=(kc == 0), stop=(kc == 1),
                        )
                    if last:
                        # tail chunks: the Scalar engine is idle after
                        # the final exp and PSUM is its fast path.
                        nc.scalar.copy(ot[:, ts(ncol, ICW)], op_ps[:])
                    else:
                        nc.vector.tensor_copy(ot[:, ts(ncol, ICW)], op_ps[:])
                eng = (nc.sync, nc.gpsimd)[o % 2] if last else nc.sync
                eng.dma_start(out_r[tt][:], ot[:])

            import functools as _ft

            def qk_ch_steps(tch, ch):
                """One channel chunk's q/k chains as ~0.9us micro-steps.
                Channel 1 is only read from att(tch) hp=1, so it can
                ride one phase later than channel 0."""
                out = []
                for wsb, dstT in ((wq_sb, qT), (wk_sb, kT)):
                    for half in range(2):
                        out.append((
                            850,
                            _ft.partial(qk_emit, wsb, dstT, ch,
                                        tch, half)))
                return out

            def v_steps(tch):
                return [(860, _ft.partial(v_emit, tch * 4 + o))
                        for o in range(4)]

            def op_steps(ic):
                return [(860, _ft.partial(op_emit, ic, o))
                        for o in range(4)]

            # ---------------- attention --------------------------
            deferred = []

            def attention(ic, fillers):
                njt = (ic + 1) * 4
                nslots = 2 * njt
                fillers = list(fillers)
                # normalize suffixes deferred from the previous chunk
                # run first (out-projection fillers read their aoT).
                for fn in deferred:
                    # a few slots in: the reciprocal-broadcast DMA the
                    # suffix reads must land first, or its wait blocks
                    # the DVE queue behind it.
                    fillers.insert(min(4, len(fillers)), (200, fn))
                deferred.clear()
                fill_total = sum(n for n, _ in fillers) or 1.0
                state = {"slot": 0, "fill": 0.0}

                def run_next():
                    n, fn = fillers.pop(0)
                    state["fill"] += n
                    fn()

                def pop_fillers():
                    state["slot"] += 1
                    target = fill_total * state["slot"] / nslots
                    while fillers and state["fill"] < target:
                        run_next()

                for hp in range(2):       # head pair = channel chunk
                    o_ps = [
                        psC.tile([65, ICW], dt32, tag=f"o{s}",
                                 name=f"o_ps{s}")
                        for s in range(2)
                    ]
                    pending_av = None
                    for jt in range(njt):
                        # diagonal tiles: only i >= j is reachable;
                        # skip the fully-masked left part.
                        r = jt - ic * 4
                        off = max(r, 0) * P
                        pt_ps = psPT.tile(
                            [P, 2 * ICW], dt32, tag="pt", name="pt_ps")
                        pt_sb = stage.tile(
                            [P, 2 * ICW], dtb, tag="pt_sb", name="pt_sb")
                        # the two heads of the pair run concurrently in
                        # the PE array (row tiling: contract=64 each).
                        for s in range(2):
                            nc.tensor.matmul(
                                pt_ps[:, s * ICW + off:(s + 1) * ICW],
                                kT[ts(s, 64), hp, ts(jt, P)],
                                qT[ts(s, 64), hp,
                                   ic * ICW + off:(ic + 1) * ICW],
                                start=True, stop=True,
                            )
                        pt_ps3 = pt_ps[:].rearrange("p (s w) -> p s w", s=2)
                        pt_sb3 = pt_sb[:].rearrange("p (s w) -> p s w", s=2)
                        nc.scalar.activation(
                            pt_sb3[:, :, off:], pt_ps3[:, :, off:],
                            MM.Exp, scale=SCALE,
                        )
                        if r >= 0:
                            # causal tri mask on the diagonal block, in
                            # place on gpsimd: keep i >= j, else 0
                            for s in range(2):
                                nc.gpsimd.affine_select(
                                    out=pt_sb[:, s * ICW + off:
                                              s * ICW + off + P],
                                    in_=pt_sb[:, s * ICW + off:
                                              s * ICW + off + P],
                                    compare_op=ALU.is_ge, fill=0.0,
                                    base=0, pattern=[[1, P]],
                                    channel_multiplier=-1,
                                )
                        # fillers first: they are dependency-free, so the
                        # PE stays busy while exp(jt-1) finishes; only
                        # then the AV whose wait would block the queue.
                        pop_fillers()
                        if pending_av is not None:
                            pending_av()

                        def av(jt=jt, off=off, pt_sb=pt_sb, hp=hp,
                               o_ps=o_ps, njt=njt):
                            for s in range(2):
                                h = 2 * hp + s
                                nc.tensor.matmul(
                                    o_ps[s][:, off:],
                                    v_sb[:, jt, h, :],
                                    pt_sb[:, s * ICW + off:(s + 1) * ICW],
                                    start=(jt == 0),
                                    stop=(jt == njt - 1),
                                )
                        pending_av = av
                    pending_av()

                    # boundary: evacuate the AV accumulators into SBUF
                    # staging right away (frees the PSUM banks for the
                    # next head-pair's AVs) and compute the reciprocals;
                    # the broadcast + normalize multiply are DEFERRED
                    # into the next stream so the PE FIFO never stalls
                    # on the DVE chain.
                    final_hp = ic == 3 and hp == 1
                    sts, recbs = [], []
                    for s in range(2):
                        if not final_hp:
                            st = stage.tile(
                                [P, ICW], dt32, tag=f"st{s}", name="st")
                            nc.vector.tensor_copy(
                                st[ts(s, 64), :], o_ps[s][0:64, :])
                            sts.append(st)
                        # denominator row to SBUF, reciprocal on DVE.
                        # At the final boundary the exp stream is done,
                        # so s=1's copy rides the idle Scalar engine and
                        # the serial DVE chain shortens.
                        den = stage.tile(
                            [1, ICW], dt32, tag="den", name="den")
                        if final_hp and s == 1:
                            nc.scalar.copy(den[:], o_ps[s][64:65, :])
                        else:
                            nc.vector.tensor_copy(den[:], o_ps[s][64:65, :])
                        rec = stage.tile(
                            [1, ICW], dt32, tag="rec", name="rec")
                        nc.vector.reciprocal_approx_fast(rec[:], den[:])
                        recb = stage.tile(
                            [1, ICW], dtb, tag=f"recb{s}", name="recb")
                        nc.vector.tensor_copy(recb[:], rec[:])
                        recbs.append(recb)

                    if final_hp:
                        # final boundary: drain whatever fillers remain
                        # (the hp=0 suffix among them), then broadcast
                        # the reciprocals with ones-matmuls and
                        # normalize in 128-col chunks, each feeding its
                        # output-projection unit immediately so the tail
                        # pipeline (DVE mul -> PE matmuls -> ACT copy ->
                        # DMA) stays full.
                        while fillers:
                            run_next()
                        # junk-matmul bridge on a psPT-pool tile (its
                        # previous user finished long ago, so no WAR
                        # delay): keeps the HAM clock warm through the
                        # ~3us reciprocal chain so the bc matmuls and
                        # out-projections run at full clock.
                        br_ps = psPT.tile([P, 2 * ICW], dt32, tag="pt",
                                          name="br_ps")
                        for _w in range(40):
                            nc.tensor.matmul(
                                br_ps[:, 0:P], junk[:], junk[:],
                                start=True, stop=True,
                            )
                        bcs_f = []
                        for s in range(2):
                            bc_ps = psPT.tile(
                                [P, 2 * ICW], dt32, tag="pt",
                                name="bc_fin")
                            nc.tensor.matmul(
                                bc_ps[0:64, 0:ICW], ones_row[:],
                                recbs[s][:], start=True, stop=True,
                            )
                            bc_sb = stage.tile(
                                [P, ICW], dtb, tag=f"bc{s}", name="bc_sb")
                            nc.vector.tensor_copy(
                                bc_sb[ts(s, 64), :], bc_ps[0:64, 0:ICW])
                            bcs_f.append(bc_sb)
                        # second bridge stretch: the broadcast casts and
                        # first normalize muls are ~3us of DVE before the
                        # first out-projection matmul can start.
                        for _w in range(56):
                            nc.tensor.matmul(
                                br_ps[:, 0:P], junk[:], junk[:],
                                start=True, stop=True,
                            )
                        for o in range(4):
                            cs = slice(o * P, (o + 1) * P)
                            for s in range(2):
                                ao_slice = aoT[ts(s, 64), hp,
                                               ic * ICW + o * P:
                                               ic * ICW + (o + 1) * P]
                                if s == 0:
                                    nc.vector.tensor_mul(
                                        ao_slice, o_ps[s][0:64, cs],
                                        bcs_f[s][0:64, cs])
                                else:
                                    nc.vector.tensor_copy(
                                        ao_slice, o_ps[s][0:64, cs])
                                    nc.vector.tensor_mul(
                                        ao_slice, ao_slice,
                                        bcs_f[s][64:128, cs])
                            op_emit(3, o, last=True)
                    else:
                        # partition-broadcast by a stride-0 DMA round
                        # trip on the (lightly loaded) gpsimd queue,
                        # issued NOW so the transfer overlaps the
                        # deferred window.
                        bcs = []
                        for s in range(2):
                            rec_d = dram_pool.tile(
                                [1, ICW], dtb, name="rec_d")
                            nc.gpsimd.dma_start(rec_d[:], recbs[s][:])
                            bc_sb = stage.tile(
                                [P, ICW], dtb, tag=f"bc{s}", name="bc_sb")
                            nc.gpsimd.dma_start(
                                bc_sb[ts(s, 64), :],
                                rec_d[0:1, :].to_broadcast((64, ICW)),
                            )
                            bcs.append(bc_sb)

                        def suffix(ic=ic, hp=hp, sts=sts, bcs=bcs):
                            for s in range(2):
                                nc.vector.tensor_mul(
                                    aoT[ts(s, 64), hp, ts(ic, ICW)],
                                    sts[s][ts(s, 64), :],
                                    bcs[s][ts(s, 64), :],
                                )

                        if hp == 0:
                            fillers.insert(min(4, len(fillers)),
                                           (200, suffix))
                        else:
                            deferred.append(suffix)
                # drain stragglers before the next phase's stream
                while fillers:
                    run_next()

            # ---------------- top-level schedule -----------------
            # only the two q/k chains attention(0) hp=0 needs run
            # up-front; the v units ride at the head of the filler
            # stream (AV(0) only needs them one slot later), so the
            # first exp fires as early as possible.
            for half in range(2):
                qk_emit(wq_sb, qT, 0, 0, half)
            for half in range(2):
                qk_emit(wk_sb, kT, 0, 0, half)

            f0 = [(50, wv_dma_step)]
            f0 += v_steps(0)[:2]
            f0 += [(50, _ft.partial(x_dma_step, 1, 0))]
            f0 += v_steps(0)[2:]
            f0 += [(50, _ft.partial(x_dma_step, 1, 1))]
            for ch in range(1, 2):
                for wsb, dstT in ((wq_sb, qT), (wk_sb, kT)):
                    for half in range(2):
                        f0.append((850, _ft.partial(
                            qk_emit, wsb, dstT, ch, 0, half)))
            f0 += [(50, _ft.partial(x_dma_step, 2, 0)),
                   (50, _ft.partial(wo_dma_step, 0))]
            f0 += qk_ch_steps(1, 0) + v_steps(1)
            f0 += [(50, _ft.partial(x_dma_step, 2, 1)),
                   (50, _ft.partial(wo_dma_step, 1))]
            f1 = qk_ch_steps(1, 1)
            f1 += [(50, _ft.partial(x_dma_step, 3, half))
                   for half in range(2)]
            f1 += qk_ch_steps(2, 0) + v_steps(2) + op_steps(0)
            attention(0, f0)
            attention(1, f1)
            attention(2, qk_ch_steps(2, 1) + qk_ch_steps(3, 0)
                      + v_steps(3) + op_steps(1))
            attention(3, qk_ch_steps(3, 1) + op_steps(2))
    nc.finalize()
    return nc


_CACHE = {}


def _get_nc():
    if "nc" not in _CACHE:
        _CACHE["nc"] = build()
    return _CACHE["nc"]


def _pack_w(w):
    # [C, F] -> [P, CC, F]: partition-major with contiguous per-
    # partition lines so the device DMA is a straight streaming read.
    f = w.shape[1]
    return np.ascontiguousarray(w.reshape(CC, P, f).transpose(1, 0, 2))


def make_in_maps(x, m, w_qkv, w_out):
    bf = ml_dtypes.bfloat16
    in_maps = []
    for core in range(8):
        b, g = divmod(core, 4)
        xt = np.asarray(x[b]).T.astype(bf)          # [C, T]
        xt = xt.reshape(CC, P, NIC, ICW).transpose(1, 2, 0, 3)
        wo = w_out[g * LC:(g + 1) * LC, :].astype(bf)  # [LC, C]
        wo = wo.reshape(2, P, C).transpose(1, 0, 2)
        in_maps.append({
            "xt": np.ascontiguousarray(xt),          # [P, NIC, CC, ICW]
            "wq": _pack_w(w_qkv[:, g * LC:(g + 1) * LC].astype(bf)),
            "wk": _pack_w(w_qkv[:, C + g * LC: C + (g + 1) * LC].astype(bf)),
            "wv": _pack_w(w_qkv[:, 2 * C + g * LC: 2 * C + (g + 1) * LC]
                          .astype(bf)),
            "wo": np.ascontiguousarray(wo),          # [P, 2, C]
            "m": np.ascontiguousarray(m[b, :, 0]).astype(np.float32),
        })
    return in_maps


def gather(results, m, b_out, B):
    out = np.zeros((B, T, C), dtype=np.float32)
    for core in range(8):
        b = core // 4
        out[b] += results[core]["out"].astype(np.float32)
    out = (out + np.asarray(b_out)[None, None, :]) * np.asarray(m)
    return out.astype(np.float32)


def kernel(x, m, w_qkv, w_out, b_out):
    x = np.asarray(x)
    m = np.asarray(m)
    in_maps = make_in_maps(x, m, np.asarray(w_qkv), np.asarray(w_out))
    nc = _get_nc()
    res = run_bass_kernel_spmd(nc, in_maps, core_ids=list(range(8)))
    return gather(res.results, m, b_out, x.shape[0])
